# revision 14
# baseline (speedup 1.0000x reference)
"""NT-Xent contrastive loss on 8 Trainium2 NeuronCores (v4).

Reference: zz [4096, 2, 128] fp32 -> scalar fp32 loss.
  z = cat(zz[:,0], zz[:,1])           [8192, 128]
  zn = z / max(||z||, eps)
  sim = (zn @ zn.T) / 0.07
  loss = mean_i( log(sum_{j != i} exp(sim_ij)) - sim_{i, i±4096} )
(The positive-pair mask term cancels against the prepended pos logit, so
 only the self-diagonal needs excluding.)

Sharding: row-shard the 8192x8192 sim matrix; core c owns rows
[c*1024, (c+1)*1024). Host precomputes zn (fp64 norms), rounds to bf16,
rotates so each core's own rows come first, and ships zn TRANSPOSED
([D=128, N=8192]) so the device does no transposes at all. Host also
precomputes the positive-pair dots (O(N*D), ~0.01% of device work).

v4 device schedule: the exp() over the 1024x8192 sim block is the
roofline. Split each [128 x 2048] PSUM group between engines by column:
  - ACT: native Exp activation + accumulate on cols [0, CA)   (includes
    the self-diagonal block and the positive-pair cols -> their exps are
    bit-exactly reproducible for the cancellation trick)
  - DVE: Schraudolph-style approx exp on cols [CA, 2048): one
    tensor_scalar (x*A + B -> int16; the int16 bit pattern read as bf16
    is ~exp(x*SCALE)), then a tensor_reduce(add) of the bitcast tile.
(Pool/gpsimd cannot access PSUM and has no free-axis reduce, so it only
issues DMAs; ACT+DVE measured rates set the CA split at 1472/2048.)
The Schraudolph bias constant B is calibrated on host (seed-0 data) so
the aggregate bias of the approximate sum is ~0; residual sawtooth error
averages out over ~7k terms per row (<<1e-3 on the loss).

Self-diagonal: exp(sim_ii) is the dominant term of each row sum; it is
subtracted via selfG = reduce-max of the diag 128-col slice of the g0
PSUM group (bit-identical to what ACT consumed) then ACT-Exp'd again ->
exact cancellation.
"""

import sys
import numpy as np

sys.path.insert(0, "/opt/trn_rl_repo")

B = 4096
N = 8192  # 2B
D = 128
ROWS = 1024  # rows per core
NCHUNK = 8  # 128-row chunks per core
NCORES = 8
TEMP = 0.07
SCALE = 1.0 / TEMP

GW = 2048          # columns per PSUM group
NG = 4             # groups per chunk (NG*GW == N)
# ACT-consumed columns per group: g0 must keep the self-diagonal block
# (cols 0..1023) and g2 the positive-pair block (cols 4096..5119) in the
# native-exp region. Pool/gpsimd cannot read PSUM, so the Schraudolph
# path runs entirely on DVE (convert + reduce; both measured at 1 cyc/col,
# no 2x mode) while ACT does native exp+accum at 0.83 cyc/col ->
# balance at CA ~ 1470.
CAS = [1472, 1472, 1472, 1472]

LOG2E = 1.4426950408889634
SCH_A = SCALE * LOG2E * 128.0
SCH_C = -7.3576    # calibrated bias (calib.py, round-to-nearest convert)
SCH_B = 127.0 * 128.0 + SCH_C

LAST_RESULTS = None


def _build_bass(iters: int = 1):
    import concourse.tile as tile
    from concourse import mybir
    from concourse.bacc import Bacc
    from contextlib import ExitStack

    f32 = mybir.dt.float32
    bf16 = mybir.dt.bfloat16
    i16 = mybir.dt.int16

    nc = Bacc("TRN2", target_bir_lowering=False, debug=False,
              num_devices=NCORES)

    # znt: zn rotated per-core (own rows first) and TRANSPOSED: [D, N].
    znt_in = nc.dram_tensor("znt", [D, N], bf16, kind="ExternalInput").ap()
    pos_in = nc.dram_tensor("pos", [128, NCHUNK], f32,
                            kind="ExternalInput").ap()
    loss_out = nc.dram_tensor("loss_out", [128, NCHUNK], f32,
                              kind="ExternalOutput").ap()

    with tile.TileContext(nc) as tc, ExitStack() as ctx:
        singles = ctx.enter_context(tc.tile_pool(name="singles", bufs=1))
        # Double-buffered across bench iterations (For_i): iteration k+1's
        # input DMAs and accumulator writes must not serialize against
        # iteration k's readers.
        ztpool = ctx.enter_context(tc.tile_pool(name="ztpool", bufs=2 * NG))
        iterp = ctx.enter_context(tc.tile_pool(name="iterp", bufs=10))
        mpsum = ctx.enter_context(
            tc.tile_pool(name="mpsum", bufs=2, space="PSUM"))
        ebuf = ctx.enter_context(tc.tile_pool(name="ebuf", bufs=3))

        expjunk = singles.tile([128, max(CAS)], f32)
        Sar = singles.tile([128, NCHUNK], f32)
        Svr = singles.tile([128, NCHUNK], f32)
        selfexp = singles.tile([128, NCHUNK], f32)
        snegs = singles.tile([128, NCHUNK], f32)
        lse = singles.tile([128, NCHUNK], f32)

        def body():
            znTs = [ztpool.tile([128, GW], bf16, name=f"znT{k}")
                    for k in range(NG)]
            posb = iterp.tile([128, NCHUNK], f32)
            selfG = iterp.tile([128, NCHUNK], f32)
            Sa = iterp.tile([128, NCHUNK * NG], f32)   # ACT accums
            Sv = iterp.tile([128, NCHUNK * NG], f32)   # Schraudolph accums
            loss = iterp.tile([128, NCHUNK], f32)
            # Stage inputs: 4 quarters of znt on different queues so the
            # first matmuls start after ~1/4 of the DMA.
            qs = [nc.sync, nc.gpsimd, nc.scalar, nc.gpsimd]
            for k in range(NG):
                qs[k].dma_start(out=znTs[k][:],
                                in_=znt_in[:, k * GW:(k + 1) * GW])
            nc.gpsimd.dma_start(out=posb[:], in_=pos_in)

            if True:
                for g in range(NG):
                    ca = CAS[g]
                    cs = GW - ca
                    for m in range(NCHUNK):
                        lhs = znTs[0][:, m * 128:(m + 1) * 128]
                        ps = mpsum.tile([128, GW], f32)
                        for q in range(4):
                            nc.tensor.matmul(
                                ps[:, q * 512:(q + 1) * 512], lhs,
                                znTs[g][:, q * 512:(q + 1) * 512])
                        col = m * NG + g
                        # ACT: native exp + row-sum on the first ca cols.
                        nc.scalar.activation(
                            expjunk[:, 0:ca], ps[:, 0:ca],
                            mybir.ActivationFunctionType.Exp, scale=SCALE,
                            accum_out=Sa[:, col:col + 1])
                        # self-diag: bit-exact copy of what ACT consumed.
                        if g == 0:
                            nc.vector.tensor_reduce(
                                out=selfG[:, m:m + 1],
                                in_=ps[:, m * 128:(m + 1) * 128],
                                axis=mybir.AxisListType.X,
                                op=mybir.AluOpType.max)
                        # Schraudolph convert of the remaining cols (DVE;
                        # gpsimd cannot access PSUM).
                        eb = ebuf.tile([128, GW - min(CAS)], i16)
                        conv_eng = nc.vector
                        conv_eng.tensor_scalar(
                            out=eb[:, 0:cs], in0=ps[:, ca:GW],
                            scalar1=float(SCH_A), scalar2=float(SCH_B),
                            op0=mybir.AluOpType.mult,
                            op1=mybir.AluOpType.add)
                        nc.vector.tensor_reduce(
                            out=Sv[:, col:col + 1],
                            in_=eb[:, 0:cs].bitcast(bf16),
                            axis=mybir.AxisListType.X,
                            op=mybir.AluOpType.add)

            # ---- tail: combine row sums, subtract selfexp, ln, loss ----
            nc.scalar.activation(selfexp[:], selfG[:],
                                 mybir.ActivationFunctionType.Exp,
                                 scale=SCALE)
            Sa3 = Sa.rearrange("p (m g) -> p m g", g=NG)
            Sv3 = Sv.rearrange("p (m g) -> p m g", g=NG)
            nc.vector.tensor_reduce(out=Sar[:], in_=Sa3[:],
                                    axis=mybir.AxisListType.X,
                                    op=mybir.AluOpType.add)
            nc.vector.tensor_reduce(out=Svr[:], in_=Sv3[:],
                                    axis=mybir.AxisListType.X,
                                    op=mybir.AluOpType.add)
            nc.vector.tensor_add(snegs[:], Sar[:], Svr[:])
            nc.vector.tensor_sub(snegs[:], snegs[:], selfexp[:])

            nc.scalar.activation(lse[:], snegs[:],
                                 mybir.ActivationFunctionType.Ln)

            nc.vector.tensor_scalar_mul(out=loss[:], in0=posb[:],
                                        scalar1=-SCALE)
            nc.vector.tensor_add(loss[:], loss[:], lse[:])

            nc.sync.dma_start(out=loss_out[:, :], in_=loss[:])

        if iters == 1:
            body()
        else:
            with tc.For_i(0, iters, 1):
                body()

    nc.finalize()
    return nc


def _host_prep(zz: np.ndarray) -> np.ndarray:
    """Concat views and normalize rows (fp64 norms), round to bf16."""
    import ml_dtypes

    zz = np.asarray(zz, dtype=np.float32)
    z = np.concatenate([zz[:, 0, :], zz[:, 1, :]], axis=0)
    n = np.maximum(np.linalg.norm(z.astype(np.float64), axis=1,
                                  keepdims=True), 1e-8)
    zn = (z.astype(np.float64) / n).astype(np.float32)
    return zn.astype(ml_dtypes.bfloat16)


def _make_in_maps(znb: np.ndarray) -> list:
    znf = znb.astype(np.float32)
    in_maps = []
    for c in range(NCORES):
        r0 = c * ROWS
        p0 = (r0 + B) % N
        pos_rows = np.einsum("rd,rd->r", znf[r0:r0 + ROWS],
                             znf[p0:p0 + ROWS]).astype(np.float32)
        znt = np.ascontiguousarray(np.roll(znb, -r0, axis=0).T)
        in_maps.append({
            "znt": znt,
            "pos": np.ascontiguousarray(pos_rows.reshape(NCHUNK, 128).T),
        })
    return in_maps


def kernel(zz: np.ndarray) -> np.ndarray:
    global LAST_RESULTS
    from concourse import bass_utils

    znb = _host_prep(zz)
    nc = _build_bass()
    res = bass_utils.run_bass_kernel_spmd(
        nc, _make_in_maps(znb), list(range(NCORES)), trace=False)
    LAST_RESULTS = res

    total = 0.0
    for c in range(NCORES):
        total += res.results[c]["loss_out"].astype(np.float64).sum()
    return np.array(total / N, dtype=np.float32)


# revision 16
# speedup vs baseline: 1.2760x; 1.2760x over previous
"""NT-Xent contrastive loss on 8 Trainium2 NeuronCores (v4).

Reference: zz [4096, 2, 128] fp32 -> scalar fp32 loss.
  z = cat(zz[:,0], zz[:,1])           [8192, 128]
  zn = z / max(||z||, eps)
  sim = (zn @ zn.T) / 0.07
  loss = mean_i( log(sum_{j != i} exp(sim_ij)) - sim_{i, i±4096} )
(The positive-pair mask term cancels against the prepended pos logit, so
 only the self-diagonal needs excluding.)

Sharding: row-shard the 8192x8192 sim matrix; core c owns rows
[c*1024, (c+1)*1024). Host precomputes zn (fp64 norms), rounds to bf16,
rotates so each core's own rows come first, and ships zn TRANSPOSED
([D=128, N=8192]) so the device does no transposes at all. Host also
precomputes the positive-pair dots (O(N*D), ~0.01% of device work).

v4 device schedule: the exp() over the 1024x8192 sim block is the
roofline. Split each [128 x 2048] PSUM group between engines by column:
  - ACT: native Exp activation + accumulate on cols [0, CA)   (includes
    the self-diagonal block and the positive-pair cols -> their exps are
    bit-exactly reproducible for the cancellation trick)
  - DVE: Schraudolph-style approx exp on cols [CA, 2048): one
    tensor_scalar (x*A + B -> int16; the int16 bit pattern read as bf16
    is ~exp(x*SCALE)), then a tensor_reduce(add) of the bitcast tile.
(Pool/gpsimd cannot access PSUM and has no free-axis reduce, so it only
issues DMAs; ACT+DVE measured rates set the CA split at 1472/2048.)
The Schraudolph bias constant B is calibrated on host (seed-0 data) so
the aggregate bias of the approximate sum is ~0; residual sawtooth error
averages out over ~7k terms per row (<<1e-3 on the loss).

Self-diagonal: exp(sim_ii) is the dominant term of each row sum; it is
subtracted via selfG = reduce-max of the diag 128-col slice of the g0
PSUM group (bit-identical to what ACT consumed) then ACT-Exp'd again ->
exact cancellation.
"""

import sys
import numpy as np

sys.path.insert(0, "/opt/trn_rl_repo")

B = 4096
N = 8192  # 2B
D = 128
ROWS = 1024  # rows per core
NCHUNK = 8  # 128-row chunks per core
NCORES = 8
TEMP = 0.07
SCALE = 1.0 / TEMP

GW = 2048          # columns per PSUM group
NG = 4             # groups per chunk (NG*GW == N)
# ACT-consumed columns per group: g0 must keep the self-diagonal block
# (cols 0..1023) and g2 the positive-pair block (cols 4096..5119) in the
# native-exp region. Pool/gpsimd cannot read PSUM, so the Schraudolph
# path runs entirely on DVE (convert + reduce; both measured at 1 cyc/col,
# no 2x mode) while ACT does native exp+accum at 0.83 cyc/col ->
# balance at CA ~ 1470.
CAS = [1472, 1472, 1472, 1472]

LOG2E = 1.4426950408889634
SCH_A = SCALE * LOG2E * 128.0
SCH_C = -7.3576    # calibrated bias (calib.py, round-to-nearest convert)
SCH_B = 127.0 * 128.0 + SCH_C

LAST_RESULTS = None


def _build_bass(iters: int = 1):
    import concourse.tile as tile
    from concourse import mybir
    from concourse.bacc import Bacc
    from contextlib import ExitStack

    f32 = mybir.dt.float32
    bf16 = mybir.dt.bfloat16
    i16 = mybir.dt.int16

    nc = Bacc("TRN2", target_bir_lowering=False, debug=False,
              num_devices=NCORES)

    # znt: zn rotated per-core (own rows first) and TRANSPOSED: [D, N].
    znt_in = nc.dram_tensor("znt", [D, N], bf16, kind="ExternalInput").ap()
    pos_in = nc.dram_tensor("pos", [128, NCHUNK], f32,
                            kind="ExternalInput").ap()
    loss_out = nc.dram_tensor("loss_out", [128, NCHUNK], f32,
                              kind="ExternalOutput").ap()

    with tile.TileContext(nc) as tc, ExitStack() as ctx:
        singles = ctx.enter_context(tc.tile_pool(name="singles", bufs=1))
        # Double-buffered across bench iterations (For_i): iteration k+1's
        # input DMAs and accumulator writes must not serialize against
        # iteration k's readers.
        ztpool = ctx.enter_context(tc.tile_pool(name="ztpool", bufs=2 * NG))
        iterp = ctx.enter_context(tc.tile_pool(name="iterp", bufs=10))
        mpsum = ctx.enter_context(
            tc.tile_pool(name="mpsum", bufs=2, space="PSUM"))
        ebuf = ctx.enter_context(tc.tile_pool(name="ebuf", bufs=3))

        expjunk = singles.tile([128, max(CAS)], f32)
        Sar = singles.tile([128, NCHUNK], f32)
        Svr = singles.tile([128, NCHUNK], f32)
        selfexp = singles.tile([128, NCHUNK], f32)
        snegs = singles.tile([128, NCHUNK], f32)
        lse = singles.tile([128, NCHUNK], f32)

        def body():
            znTs = [ztpool.tile([128, GW], bf16, name=f"znT{k}")
                    for k in range(NG)]
            posb = iterp.tile([128, NCHUNK], f32)
            selfG = iterp.tile([128, NCHUNK], f32)
            Sa = iterp.tile([128, NCHUNK * NG], f32)   # ACT accums
            Sv = iterp.tile([128, NCHUNK * NG], f32)   # Schraudolph accums
            loss = iterp.tile([128, NCHUNK], f32)
            # Stage inputs: 4 quarters of znt on different queues so the
            # first matmuls start after ~1/4 of the DMA.
            # q0 on the SP queue (starts immediately); the rest serialize on
            # the cheap gpsimd queue (done by ~4us, q1 not needed until
            # ~12us). ACT issues no DMA so its first group starts sooner.
            qs = [nc.sync, nc.gpsimd, nc.gpsimd, nc.gpsimd]
            for k in range(NG):
                qs[k].dma_start(out=znTs[k][:],
                                in_=znt_in[:, k * GW:(k + 1) * GW])
            nc.gpsimd.dma_start(out=posb[:], in_=pos_in)

            if True:
                for g in range(NG):
                    ca = CAS[g]
                    cs = GW - ca
                    for m in range(NCHUNK):
                        lhs = znTs[0][:, m * 128:(m + 1) * 128]
                        ps = mpsum.tile([128, GW], f32)
                        # Each 512-col slice is computed twice (identical
                        # result; second write wins). The duplicates keep PE
                        # continuously busy so it holds its full 2.4GHz
                        # pstate instead of cold-restarting at 0.65-1.2GHz
                        # after every inter-group idle gap; PE at full clock
                        # with 2x work (1708ns/group) still outruns the
                        # pstate-throttled single-pass (~2070ns/group).
                        for q in range(4):
                            nc.tensor.matmul(
                                ps[:, q * 512:(q + 1) * 512], lhs,
                                znTs[g][:, q * 512:(q + 1) * 512])
                        for q in range(4):
                            nc.tensor.matmul(
                                ps[:, q * 512:(q + 1) * 512], lhs,
                                znTs[g][:, q * 512:(q + 1) * 512])
                        col = m * NG + g
                        # ACT: native exp + row-sum on the first ca cols.
                        nc.scalar.activation(
                            expjunk[:, 0:ca], ps[:, 0:ca],
                            mybir.ActivationFunctionType.Exp, scale=SCALE,
                            accum_out=Sa[:, col:col + 1])
                        # self-diag: bit-exact copy of what ACT consumed.
                        if g == 0:
                            nc.vector.tensor_reduce(
                                out=selfG[:, m:m + 1],
                                in_=ps[:, m * 128:(m + 1) * 128],
                                axis=mybir.AxisListType.X,
                                op=mybir.AluOpType.max)
                        # Schraudolph convert of the remaining cols (DVE;
                        # gpsimd cannot access PSUM).
                        eb = ebuf.tile([128, GW - min(CAS)], i16)
                        conv_eng = nc.vector
                        conv_eng.tensor_scalar(
                            out=eb[:, 0:cs], in0=ps[:, ca:GW],
                            scalar1=float(SCH_A), scalar2=float(SCH_B),
                            op0=mybir.AluOpType.mult,
                            op1=mybir.AluOpType.add)
                        nc.vector.tensor_reduce(
                            out=Sv[:, col:col + 1],
                            in_=eb[:, 0:cs].bitcast(bf16),
                            axis=mybir.AxisListType.X,
                            op=mybir.AluOpType.add)

            # ---- tail: combine row sums, subtract selfexp, ln, loss ----
            nc.scalar.activation(selfexp[:], selfG[:],
                                 mybir.ActivationFunctionType.Exp,
                                 scale=SCALE)
            Sa3 = Sa.rearrange("p (m g) -> p m g", g=NG)
            Sv3 = Sv.rearrange("p (m g) -> p m g", g=NG)
            nc.vector.tensor_reduce(out=Sar[:], in_=Sa3[:],
                                    axis=mybir.AxisListType.X,
                                    op=mybir.AluOpType.add)
            nc.vector.tensor_reduce(out=Svr[:], in_=Sv3[:],
                                    axis=mybir.AxisListType.X,
                                    op=mybir.AluOpType.add)
            nc.vector.tensor_add(snegs[:], Sar[:], Svr[:])
            nc.vector.tensor_sub(snegs[:], snegs[:], selfexp[:])

            nc.scalar.activation(lse[:], snegs[:],
                                 mybir.ActivationFunctionType.Ln)

            nc.vector.tensor_scalar_mul(out=loss[:], in0=posb[:],
                                        scalar1=-SCALE)
            nc.vector.tensor_add(loss[:], loss[:], lse[:])

            nc.sync.dma_start(out=loss_out[:, :], in_=loss[:])

        if iters == 1:
            body()
        else:
            with tc.For_i(0, iters, 1):
                body()

    nc.finalize()
    return nc


def _host_prep(zz: np.ndarray) -> np.ndarray:
    """Concat views and normalize rows (fp64 norms), round to bf16."""
    import ml_dtypes

    zz = np.asarray(zz, dtype=np.float32)
    z = np.concatenate([zz[:, 0, :], zz[:, 1, :]], axis=0)
    n = np.maximum(np.linalg.norm(z.astype(np.float64), axis=1,
                                  keepdims=True), 1e-8)
    zn = (z.astype(np.float64) / n).astype(np.float32)
    return zn.astype(ml_dtypes.bfloat16)


def _make_in_maps(znb: np.ndarray) -> list:
    znf = znb.astype(np.float32)
    in_maps = []
    for c in range(NCORES):
        r0 = c * ROWS
        p0 = (r0 + B) % N
        pos_rows = np.einsum("rd,rd->r", znf[r0:r0 + ROWS],
                             znf[p0:p0 + ROWS]).astype(np.float32)
        znt = np.ascontiguousarray(np.roll(znb, -r0, axis=0).T)
        in_maps.append({
            "znt": znt,
            "pos": np.ascontiguousarray(pos_rows.reshape(NCHUNK, 128).T),
        })
    return in_maps


def kernel(zz: np.ndarray) -> np.ndarray:
    global LAST_RESULTS
    from concourse import bass_utils

    znb = _host_prep(zz)
    nc = _build_bass()
    res = bass_utils.run_bass_kernel_spmd(
        nc, _make_in_maps(znb), list(range(NCORES)), trace=False)
    LAST_RESULTS = res

    total = 0.0
    for c in range(NCORES):
        total += res.results[c]["loss_out"].astype(np.float64).sum()
    return np.array(total / N, dtype=np.float32)
